# revision 1
# baseline (speedup 1.0000x reference)
"""Trainium2 Bass kernel for nn_AttnAggregator (GAT-style neighbor aggregation).

Reference computation:
    ep = embed_table @ W_proj.T                      # [N, 64]
    neigh = ep[padded_neighs]                        # [B, L, 64]
    scores = leaky_relu(ep[dst]@a_dst) + leaky_relu(neigh@a_src)
    attn = masked_softmax(scores, axis=L)
    out = sum_l attn * neigh                         # [B, 64]

Key algebraic fact: the dst term is constant along the softmax axis L, so it
cancels in the softmax — the output does not depend on dst_idx / a_dst.

Strategy (8 NeuronCores, memory-bound target):
  Launch 1 (projection, table-row-sharded): each core projects N/8 = 25000
    table rows to 64 features and also precomputes es = exp(leaky_relu(ep@a_src))
    per row, emitting an extended row [64 feats | es] = 65 f32. Host
    concatenates the 8 slices into the full extended table (a pure unshard).
  Launch 2 (attention, batch-sharded): each core handles B/8 = 6250 nodes.
    Masked neighbors have exactly zero softmax weight, so the host compacts
    each node's unmasked neighbors to the front and the kernel only gathers
    up to each 128-node tile's max unmasked count (~37 of 50 columns).
    Each neighbor column is one indirect DMA (the TRN2 indirect DMA supports
    exactly one row index per partition: 128 rows of 260B per instruction);
    then on-chip: w = es*mask, den = sum_l w, num = sum_l w*feat, out = num/den.
    The launch is bound by SWDGE descriptor generation (~1.5us per indirect
    DMA on the GpSimd engine); gather bytes and DVE compute overlap under it.
    Measured: ~275us (launch 1) + ~2.63ms (launch 2) across 8 cores, rel err
    ~2e-6 vs the fp32 reference.
"""

import os
import sys

sys.path.insert(0, "/opt/trn_rl_repo")

import numpy as np

# ---- hardcoded problem shapes -------------------------------------------------
B, L, N = 50000, 50, 200000
D_IN, D_OUT = 256, 64
NCORES = 8
R = N // NCORES        # 25000 table rows per core (launch 1)
BN = B // NCORES       # 6250 batch nodes per core (launch 2)
EXT = D_OUT + 1        # 65: [64 projected feats | exp(leaky_relu(score))]
P = 128

_CACHE = {}
LAST_PERF = []         # filled when KERNEL_TRACE=1: list of BassKernelResults


def _build_proj():
    import concourse.bass as bass
    from concourse import bacc, mybir
    from concourse.tile import TileContext
    from contextlib import ExitStack

    F32 = mybir.dt.float32
    nc = bacc.Bacc("TRN2", target_bir_lowering=False)
    tT = nc.dram_tensor("tT", [D_IN, R], F32, kind="ExternalInput")
    wT = nc.dram_tensor("wT", [D_IN, D_OUT], F32, kind="ExternalInput")
    a = nc.dram_tensor("a", [P, D_OUT], F32, kind="ExternalInput")
    ep = nc.dram_tensor("ep", [R, EXT], F32, kind="ExternalOutput")

    CPB = 8  # chunks (of 128 table rows) per PSUM block; 8*64 = 512 f32 = 1 bank
    BLK = P * CPB

    with TileContext(nc) as tc, ExitStack() as ctx:
        singles = ctx.enter_context(tc.tile_pool(name="singles", bufs=1))
        tpool = ctx.enter_context(tc.tile_pool(name="tpool", bufs=3))
        stpool = ctx.enter_context(tc.tile_pool(name="stpool", bufs=3))
        bcpool = ctx.enter_context(tc.tile_pool(name="bcpool", bufs=3))
        espool = ctx.enter_context(tc.tile_pool(name="espool", bufs=3))
        spool = ctx.enter_context(tc.tile_pool(name="spool", bufs=4))
        psum = ctx.enter_context(tc.tile_pool(name="psum", bufs=2, space="PSUM"))

        w_ld = singles.tile([P, 2, D_OUT], F32)
        nc.sync.dma_start(out=w_ld[:], in_=wT.rearrange("(k p) n -> p k n", p=P))
        # stage weights through DVE so matmuls never wait on the weight DMA
        w_sb = singles.tile([P, 2, D_OUT], F32)
        nc.vector.tensor_copy(out=w_sb[:], in_=w_ld[:])
        a_sb = singles.tile([P, D_OUT], F32)
        nc.sync.dma_start(out=a_sb[:], in_=a[:, :])
        tTr = tT.rearrange("(k p) r -> p k r", p=P)

        for B0 in range(0, R, BLK):
            wcols = min(BLK, R - B0)
            nj = (wcols + P - 1) // P
            tt = tpool.tile([P, 2, BLK], F32)
            nc.sync.dma_start(out=tt[:, :, :wcols], in_=tTr[:, :, B0 : B0 + wcols])
            # staged copy of the first chunk's k0 weights: the block's first
            # matmul then waits only on DVE (stage + prior PSUM release).
            cw0 = min(P, wcols)
            st = stpool.tile([P, P], F32)
            nc.vector.tensor_copy(out=st[:, :cw0], in_=tt[:, 0, :cw0])
            ps = psum.tile([P, CPB * D_OUT], F32, space="PSUM")
            # DVE-zero the bank first: the block's first matmul then sees DVE
            # as the last writer, merging the PSUM WAW dep into its DVE wait.
            nc.vector.memset(ps[:, :], 0)
            for j in range(nj):
                c0 = B0 + j * P
                cw = min(P, R - c0)
                lhs0 = st[:, :cw] if j == 0 else tt[:, 0, j * P : j * P + cw]
                nc.tensor.matmul(
                    ps[:cw, j * D_OUT : (j + 1) * D_OUT],
                    lhs0,
                    w_sb[:, 0, :],
                    start=True,
                    stop=False,
                )
                nc.tensor.matmul(
                    ps[:cw, j * D_OUT : (j + 1) * D_OUT],
                    tt[:, 1, j * P : j * P + cw],
                    w_sb[:, 1, :],
                    start=False,
                    stop=True,
                )
            BC = bcpool.tile([P, CPB, D_OUT], F32)
            nc.vector.tensor_copy(
                out=BC[:, :nj, :], in_=ps[:, 0 : nj * D_OUT]
            )
            ES = espool.tile([P, CPB], F32)
            for j in range(nj):
                sc = spool.tile([P, 1], F32)
                scr = spool.tile([P, D_OUT], F32)
                nc.vector.tensor_tensor(
                    out=scr[:], in0=BC[:, j, :], in1=a_sb[:], op=mybir.AluOpType.mult
                )
                nc.vector.tensor_reduce(
                    out=sc[:], in_=scr[:], axis=mybir.AxisListType.X,
                    op=mybir.AluOpType.add,
                )
                lr = spool.tile([P, 1], F32)
                nc.vector.scalar_tensor_tensor(
                    out=lr[:],
                    in0=sc[:],
                    scalar=0.2,
                    in1=sc[:],
                    op0=mybir.AluOpType.mult,
                    op1=mybir.AluOpType.max,
                )
                nc.scalar.activation(
                    out=ES[:, j : j + 1],
                    in_=lr[:],
                    func=mybir.ActivationFunctionType.Exp,
                )
            if wcols == BLK:
                nc.sync.dma_start(
                    out=ep[B0 : B0 + BLK, 0:D_OUT].rearrange(
                        "(j p) d -> p j d", p=P
                    ),
                    in_=BC[:, :, :],
                )
                nc.sync.dma_start(
                    out=ep[B0 : B0 + BLK, D_OUT:EXT].rearrange(
                        "(j p) o -> p (j o)", p=P
                    ),
                    in_=ES[:, :],
                )
            else:
                for j in range(nj):
                    c0 = B0 + j * P
                    cw = min(P, R - c0)
                    nc.sync.dma_start(
                        out=ep[c0 : c0 + cw, 0:D_OUT], in_=BC[:cw, j, :]
                    )
                    nc.sync.dma_start(
                        out=ep[c0 : c0 + cw, D_OUT:EXT], in_=ES[:cw, j : j + 1]
                    )
    return nc


def _build_attn(tile_counts=None):
    import concourse.bass as bass
    from concourse import bacc, mybir
    from concourse.tile import TileContext
    from contextlib import ExitStack

    F32 = mybir.dt.float32
    I32 = mybir.dt.int32
    if tile_counts is None:
        tile_counts = [L] * ((BN + P - 1) // P)
    nc = bacc.Bacc("TRN2", target_bir_lowering=False)
    ep = nc.dram_tensor("ep", [N, EXT], F32, kind="ExternalInput")
    idx = nc.dram_tensor("idx", [BN, L], I32, kind="ExternalInput")
    mkf = nc.dram_tensor("mkf", [BN, L], F32, kind="ExternalInput")
    out = nc.dram_tensor("out", [BN, D_OUT], F32, kind="ExternalOutput")

    with TileContext(nc) as tc, ExitStack() as ctx:
        ipool = ctx.enter_context(tc.tile_pool(name="ipool", bufs=8))
        gpool = ctx.enter_context(tc.tile_pool(name="gpool", bufs=6))
        wfpool = ctx.enter_context(tc.tile_pool(name="wfpool", bufs=4))
        spool = ctx.enter_context(tc.tile_pool(name="spool", bufs=6))
        opool = ctx.enter_context(tc.tile_pool(name="opool", bufs=4))

        for ti, t0 in enumerate(range(0, BN, P)):
            p = min(P, BN - t0)
            Lc = tile_counts[ti]
            it = ipool.tile([P, L], I32)
            nc.sync.dma_start(out=it[:p, :Lc], in_=idx[t0 : t0 + p, 0:Lc])
            mt = ipool.tile([P, L], F32)
            nc.sync.dma_start(out=mt[:p, :Lc], in_=mkf[t0 : t0 + p, 0:Lc])
            G = gpool.tile([P, L, EXT], F32)
            # HW indirect DMA supports exactly one index per partition, so
            # gather one neighbor column (128 rows) per instruction. Columns
            # beyond this tile's max unmasked-neighbor count are skipped
            # entirely (host compacts unmasked neighbors to the front).
            for l in range(Lc):
                nc.gpsimd.indirect_dma_start(
                    out=G[:p, l, :],
                    out_offset=None,
                    in_=ep[:, :],
                    in_offset=bass.IndirectOffsetOnAxis(ap=it[:p, l : l + 1], axis=0),
                )
            w = spool.tile([P, L], F32)
            den = spool.tile([P, 1], F32)
            nc.vector.tensor_tensor(
                out=w[:p, :Lc], in0=G[:p, :Lc, D_OUT], in1=mt[:p, :Lc],
                op=mybir.AluOpType.mult,
            )
            nc.vector.tensor_reduce(
                out=den[:p], in_=w[:p, :Lc], axis=mybir.AxisListType.X,
                op=mybir.AluOpType.add,
            )
            WF = wfpool.tile([P, L, D_OUT], F32)
            wb = w[:p, :Lc].to_broadcast([p, Lc, D_OUT])
            nc.vector.tensor_tensor(
                out=WF[:p, :Lc, :], in0=G[:p, :Lc, 0:D_OUT], in1=wb,
                op=mybir.AluOpType.mult,
            )
            num = spool.tile([P, D_OUT], F32)
            nc.vector.tensor_reduce(
                out=num[:p],
                in_=WF[:p, :Lc, :].rearrange("p l d -> p d l"),
                axis=mybir.AxisListType.X,
                op=mybir.AluOpType.add,
            )
            r = spool.tile([P, 1], F32)
            nc.vector.reciprocal(out=r[:p], in_=den[:p])
            ot = opool.tile([P, D_OUT], F32)
            rb = r[:p].to_broadcast([p, D_OUT])
            nc.vector.tensor_tensor(
                out=ot[:p], in0=num[:p], in1=rb, op=mybir.AluOpType.mult
            )
            nc.sync.dma_start(out=out[t0 : t0 + p, :], in_=ot[:p])
    return nc


def _get_nc(key, builder):
    if key not in _CACHE:
        nc = builder()
        nc.finalize()  # Bacc.finalize runs wait-splitting/legalization passes
        _CACHE[key] = nc
    return _CACHE[key]


def kernel(
    padded_neighs,
    mask,
    dst_idx,
    embed_table,
    W_proj,
    a_src,
    a_dst,
):
    from concourse.bass_utils import run_bass_kernel_spmd

    del dst_idx, a_dst  # constant along softmax axis -> cancels exactly

    trace = bool(int(os.environ.get("KERNEL_TRACE", "0")))
    LAST_PERF.clear()

    padded_neighs = np.asarray(padded_neighs, dtype=np.int32)
    mask = np.asarray(mask, dtype=bool)
    # Masked neighbors get exactly zero softmax weight (the reference masks
    # with -1e9 -> exp underflows to 0), so skipping them is exact. Compact
    # each node's unmasked neighbors to the front; the kernel then only
    # gathers up to each tile's max unmasked count.
    order = np.argsort(~mask, axis=1, kind="stable")
    padded_neighs = np.ascontiguousarray(np.take_along_axis(padded_neighs, order, axis=1))
    maskf = np.ascontiguousarray(
        np.take_along_axis(mask, order, axis=1).astype(np.float32)
    )
    # Sort each core's nodes by descending unmasked count so tiles are
    # homogeneous: the per-tile max column count then tracks the average
    # (~26) instead of the tile max over random nodes (~37). Pure row
    # reordering: inputs are permuted here, outputs un-permuted below.
    counts = mask.sum(axis=1).reshape(NCORES, BN)
    node_order = np.argsort(-counts, axis=1, kind="stable")  # [NCORES, BN]
    counts_sorted = np.take_along_axis(counts, node_order, axis=1)
    tile_counts = tuple(
        max(1, int(counts_sorted[:, t0].max())) for t0 in range(0, BN, P)
    )
    tT = np.ascontiguousarray(np.asarray(embed_table, dtype=np.float32).T)
    wT = np.ascontiguousarray(np.asarray(W_proj, dtype=np.float32).T)
    a = np.ascontiguousarray(
        np.tile(np.asarray(a_src, dtype=np.float32)[None, :], (P, 1))
    )

    core_ids = list(range(NCORES))

    # ---- launch 1: projection (table rows sharded) ---------------------------
    nc1 = _get_nc("proj", _build_proj)
    in1 = [
        {
            "tT": np.ascontiguousarray(tT[:, c * R : (c + 1) * R]),
            "wT": wT,
            "a": a,
        }
        for c in core_ids
    ]
    res1 = run_bass_kernel_spmd(nc1, in1, core_ids=core_ids, trace=trace)
    ep = np.concatenate([r["ep"] for r in res1.results], axis=0)  # [N, EXT]

    # ---- launch 2: gather + attention (batch nodes sharded) ------------------
    nc2 = _get_nc(("attn", tile_counts), lambda: _build_attn(list(tile_counts)))
    in2 = [
        {
            "ep": ep,
            "idx": np.ascontiguousarray(
                padded_neighs[c * BN : (c + 1) * BN][node_order[c]]
            ),
            "mkf": np.ascontiguousarray(
                maskf[c * BN : (c + 1) * BN][node_order[c]]
            ),
        }
        for c in core_ids
    ]
    res2 = run_bass_kernel_spmd(nc2, in2, core_ids=core_ids, trace=trace)
    outs = []
    for c in core_ids:
        dev = res2.results[c]["out"]
        unperm = np.empty_like(dev)
        unperm[node_order[c]] = dev
        outs.append(unperm)
    out = np.concatenate(outs, axis=0)  # [B, D_OUT]

    if trace:
        LAST_PERF.extend([res1, res2])
    return np.ascontiguousarray(out, dtype=np.float32)

